# revision 32
# baseline (speedup 1.0000x reference)
"""Trainium2 Bass kernel for nn_FDC2_61108794688088.

Math: out[i, c] = BS * s1[i, c] + (W2 @ colsum)[c] + BS * b_fc[c]
  where s1 = z1 @ W_fc[:, :2048].T
        colsum = sum_j relu(z2f @ W_proj.T + b_proj)[j, :]
        W2 = W_fc[:, 2048:]

Sharding: 2D — 4 batch groups x 2 feature halves across the 8 cores.
Core (b, h) handles batch rows [512b, 512b+512) and projection output
features [512h, 512h+512); s1's contraction over the 2048 hidden dims is
split in half between the two h-cores of each batch group (partials summed
on host). Only tiny vectors cross cores, all reduced on host during the
gather: the [1024] colsum and the [65, 512] s1 partials.

The projection matmul runs in fp8 E4M3 (weights pre-scaled by 64, the 1/64
folded into the relu's scale) with DoubleRow packing (256 K-rows per
matmul); s1 runs in float32r (full fp32 precision, 1 cycle/row).

Raw Bacc (no TileContext): everything stays resident in SBUF (no pool
recycling, so no WAR hazards), each projection m-tile owns a PSUM bank,
and ordering is a handful of hand-placed semaphores.
"""

import os
import sys

import numpy as np


def _import_concourse():
    try:
        import concourse.bass  # noqa: F401
    except ImportError:
        for p in ("/opt/trn_rl_repo", "/root/.axon_site/_ro/trn_rl_repo"):
            if os.path.isdir(p) and p not in sys.path:
                sys.path.append(p)
        import concourse.bass  # noqa: F401


_import_concourse()

import ml_dtypes  # noqa: E402

import concourse.bacc as bacc  # noqa: E402
from concourse import mybir  # noqa: E402
from concourse import bass_utils  # noqa: E402

BS = 2048
HID = 2048
PIN = 3 * 56 * 56  # 9408
POUT = 1024
NCLS = 65
NCORES = 8
NB = 4  # batch groups
NH = 2  # feature halves
B = BS // NB  # 512 batch rows per core
MT = POUT // 128 // NH  # 4 m-tiles of 128 output features per core
KT2 = (PIN + 127) // 128  # 74 k-tiles for the projection (padded to 9472)
KP2 = KT2 // 2  # 37 DoubleRow k-pairs
KH = HID // NH  # 1024 hidden dims of s1 contraction per core
KT1 = KH // 128  # 8 k-tiles for s1
WSCALE = 64.0  # fp8 weight pre-scale

FP8 = ml_dtypes.float8_e4m3

_NC_CACHE = None
LAST_RESULTS = None  # BassKernelResults of the most recent run (for profiling)


def _build_nc():
    """Build the per-core Bass module (identical on all 8 cores)."""
    nc = bacc.Bacc(target_bir_lowering=False)
    dt = mybir.dt

    # The projection operands (z2 + all four wp tiles) live in ONE
    # host-interleaved fp8 stream laid out in exact consumption order, so
    # the whole load is a few large DMAs with ~22KB contiguous runs per
    # partition (the DMA engines' sweet spot). Per partition, k-chunk
    # (a, b) occupies bytes [2048a, 2048b): first the z2 pairs (1024B
    # each), then the four wp tiles' pairs (256B each).
    PAIR_Z = 2 * B  # 1024 bytes per z2 pair per partition
    PAIR_W = 2 * 128  # 256 bytes per wp pair per partition
    PAIR_BYTES = PAIR_Z + MT * PAIR_W  # 2048
    STREAM_BYTES = KP2 * PAIR_BYTES

    stream = nc.dram_tensor(
        "stream", [128, STREAM_BYTES], dt.float8e4, kind="ExternalInput"
    )
    bp = nc.dram_tensor("bp", [128, MT], dt.float32, kind="ExternalInput")
    # z1^T block and 2048*W_fc^T block fused into one tensor so the first
    # float32r matmul (self-loading, single sync-wait slot) waits on one DMA.
    zw = nc.dram_tensor("zw", [128, KT1, B + NCLS], dt.float32r, kind="ExternalInput")

    s1t_out = nc.dram_tensor("s1t", [NCLS, B], dt.float32, kind="ExternalOutput")
    colsum_out = nc.dram_tensor("colsum", [128, MT], dt.float32, kind="ExternalOutput")

    # SBUF: everything resident simultaneously (~97 KB/partition of 192).
    st_sb = nc.alloc_sbuf_tensor("st_sb", [128, STREAM_BYTES], dt.float8e4)[:]
    zw_sb = nc.alloc_sbuf_tensor("zw_sb", [128, KT1, B + NCLS], dt.float32r)[:]
    bp_sb = nc.alloc_sbuf_tensor("bp_sb", [128, MT], dt.float32)[:]
    relu_sb = nc.alloc_sbuf_tensor("relu_sb", [128, B], dt.float32)[:]
    colsum_sb = nc.alloc_sbuf_tensor("colsum_sb", [128, MT], dt.float32)[:]
    s1_sb = nc.alloc_sbuf_tensor("s1_sb", [NCLS, B], dt.float32)[:]

    def z2_ap(chunk, kp):
        a, b = chunk
        o = a * PAIR_BYTES + (kp - a) * PAIR_Z
        return st_sb[:, o : o + PAIR_Z].rearrange("p (j n) -> p j n", j=2)

    def wp_ap(chunk, t, kp):
        a, b = chunk
        o = a * PAIR_BYTES + (b - a) * PAIR_Z + t * (b - a) * PAIR_W
        o += (kp - a) * PAIR_W
        return st_sb[:, o : o + PAIR_W].rearrange("p (j n) -> p j n", j=2)

    # PSUM: one bank per m-tile (all four accumulate concurrently), plus a
    # separate bank for s1 so it can run while the activations drain ps0-3.
    ps = [nc.alloc_psum_tensor(f"ps{t}", [128, B], dt.float32)[:] for t in range(MT)]
    ps1 = nc.alloc_psum_tensor("ps1t", [NCLS, B], dt.float32)[:]

    # The byte stream arrives in k-chunks, each chunk carrying the z2 slice
    # plus all four wp slices for that k-range; the PE interleaves the four
    # m-tiles' matmuls within each chunk so every arriving byte unlocks
    # work immediately. The final chunk is processed m0..m3 sequentially so
    # the activations pipeline instead of bunching at the end.
    CHUNKS = [(0, 4), (4, 15), (15, 26), (26, KP2)]
    NC_ = len(CHUNKS)
    s_st = [nc.alloc_semaphore(f"s_st_{i}") for i in range(NC_)]
    s_bp = nc.alloc_semaphore("s_bp")
    s_zw = nc.alloc_semaphore("s_zw")
    pesem = nc.alloc_semaphore("pesem")  # +1 per finished psum group
    vsem = nc.alloc_semaphore("vsem")  # s1 psum->sbuf copy done
    qout1 = nc.alloc_semaphore("qout1")  # s1t output DMA
    qout2 = nc.alloc_semaphore("qout2")  # colsum output DMA
    donesem = nc.alloc_semaphore("donesem")
    all_sems = s_st + [s_bp, s_zw, pesem, vsem, qout1, qout2, donesem]

    with nc.Block() as block:

        @block.sync
        def _(sync):
            # single queue, consumption order; zw rides between chunk sets
            for i, (a, b) in enumerate(CHUNKS):
                sync.dma_start(
                    out=st_sb[:, a * PAIR_BYTES : b * PAIR_BYTES],
                    in_=stream[:, a * PAIR_BYTES : b * PAIR_BYTES],
                ).then_inc(s_st[i], 16)
                if i == 0:
                    sync.dma_start(out=bp_sb, in_=bp[:]).then_inc(s_bp, 16)
                elif i == 1:
                    sync.dma_start(out=zw_sb, in_=zw[:]).then_inc(s_zw, 16)
            # s1 output after the vector copy
            sync.wait_ge(vsem, 1)
            sync.dma_start(out=s1t_out[:], in_=s1_sb).then_inc(qout1, 16)
            sync.wait_ge(qout1, 16)
            sync.sem_inc(donesem, 1)

        @block.tensor
        def _(tensor):
            last = NC_ - 1
            for i, chunk in enumerate(CHUNKS):
                a, b = chunk
                tensor.wait_ge(s_st[i], 16)
                if i < last:
                    for kp in range(a, b):
                        for t in range(MT):
                            nc.tensor.matmul(
                                ps[t],
                                lhsT=wp_ap(chunk, t, kp),
                                rhs=z2_ap(chunk, kp),
                                start=(kp == 0),
                                stop=False,
                                perf_mode=mybir.MatmulPerfMode.DoubleRow,
                                skip_group_check=True,
                            )
                else:
                    # final chunk: finish m-tiles one at a time
                    for t in range(MT):
                        for kp in range(a, b):
                            mm = nc.tensor.matmul(
                                ps[t],
                                lhsT=wp_ap(chunk, t, kp),
                                rhs=z2_ap(chunk, kp),
                                start=False,
                                stop=(kp == KP2 - 1),
                                perf_mode=mybir.MatmulPerfMode.DoubleRow,
                                skip_group_check=True,
                            )
                        mm.then_inc(pesem, 1)
            # s1: 8 float32r matmuls accumulating 2048*s1^T (K-half) on its
            # own psum bank, concurrent with the activations.
            tensor.wait_ge(s_zw, 16)
            for ki in range(KT1):
                mm = nc.tensor.matmul(
                    ps1,
                    lhsT=zw_sb[:, ki, B:],
                    rhs=zw_sb[:, ki, :B],
                    start=(ki == 0),
                    stop=(ki == KT1 - 1),
                )
            mm.then_inc(pesem, 1)

        @block.scalar
        def _(scalar):
            scalar.wait_ge(s_bp, 16)
            for t in range(MT):
                scalar.wait_ge(pesem, t + 1)
                nc.scalar.activation(
                    out=relu_sb,
                    in_=ps[t],
                    func=mybir.ActivationFunctionType.Relu,
                    bias=bp_sb[:, t : t + 1],
                    scale=1.0 / WSCALE,
                    accum_out=colsum_sb[:, t : t + 1],
                )
            nc.scalar.dma_start(out=colsum_out[:], in_=colsum_sb).then_inc(qout2, 16)
            scalar.wait_ge(qout2, 16)
            scalar.sem_inc(donesem, 1)

        @block.vector
        def _(vector):
            vector.wait_ge(pesem, MT + 1)
            nc.vector.tensor_copy(out=s1_sb, in_=ps1).then_inc(vsem, 1)

        @block.gpsimd
        def _(gpsimd):
            gpsimd.wait_ge(donesem, 2)
            for sem in all_sems:
                gpsimd.sem_clear(sem)

    if not nc.is_finalized():
        nc.finalize()
    return nc


def _prep_inputs(z1, z2, W_proj, b_proj, W_fc):
    """Host-side sharding + layout. Returns per-core input maps.

    Core c = 2*b + h: batch group b (512 rows), feature half h.
    """
    z2f = np.ascontiguousarray(z2.reshape(BS, PIN))

    # z2f^T padded to [74*128, 2048] fp8; per batch group [128, 37, 2, 512]:
    # z2ft[p, t, j, n] = z2f^T[(2t+j)*128 + p, 512b + n]
    Z = np.zeros((KT2 * 128, BS), dtype=FP8)
    Z[:PIN] = z2f.T.astype(FP8)
    z2_shards = [
        np.ascontiguousarray(
            Z[:, b * B : (b + 1) * B].reshape(KP2, 2, 128, B).transpose(2, 0, 1, 3)
        )
        for b in range(NB)
    ]

    # 64 * W_proj^T padded, arranged [8, 128, 37, 2, 128]; half h is the
    # contiguous m-tile slice [4h:4h+4].
    Wp = np.zeros((KT2 * 128, POUT), dtype=FP8)
    Wp[:PIN] = (W_proj.T * np.float32(WSCALE)).astype(FP8)
    wpt_full = np.ascontiguousarray(
        Wp.reshape(KP2, 2, 128, NH * MT, 128).transpose(3, 2, 0, 1, 4)
    )
    wpt_halves = [wpt_full[h * MT : (h + 1) * MT] for h in range(NH)]

    bp_full = b_proj.reshape(NH * MT, 128).T.astype(np.float32)  # [128, 8]
    bp_halves = [
        np.ascontiguousarray(bp_full[:, h * MT : (h + 1) * MT]) for h in range(NH)
    ]

    # 2048 * W_fc[:, :HID]^T split into K-halves [128, 8, 65]
    W1sT = np.float32(BS) * W_fc[:, :HID].T.astype(np.float32)  # [2048, 65]
    w1_halves = [
        np.ascontiguousarray(
            W1sT[h * KH : (h + 1) * KH].reshape(KT1, 128, NCLS).transpose(1, 0, 2)
        )
        for h in range(NH)
    ]

    # assemble the interleaved fp8 stream per core: per k-chunk, the z2
    # pairs then the four wp tiles' pairs (matching the device AP layout)
    PAIR_Z = 2 * B
    PAIR_W = 2 * 128
    PAIR_BYTES = PAIR_Z + MT * PAIR_W
    CHUNKS = [(0, 4), (4, 15), (15, 26), (26, KP2)]

    def build_stream(z2c, wpt_h):
        st = np.empty((128, KP2 * PAIR_BYTES), dtype=FP8)
        for a, b in CHUNKS:
            base = a * PAIR_BYTES
            zlen = (b - a) * PAIR_Z
            st[:, base : base + zlen] = z2c[:, a:b].reshape(128, -1)
            for t in range(MT):
                off = base + zlen + t * (b - a) * PAIR_W
                st[:, off : off + (b - a) * PAIR_W] = wpt_h[t][:, a:b].reshape(
                    128, -1
                )
        return st

    streams = {}
    in_maps = []
    for b in range(NB):
        for h in range(NH):
            if (b, h) not in streams:
                streams[(b, h)] = build_stream(z2_shards[b], wpt_halves[h])
            z1_blk = (
                z1[b * B : (b + 1) * B, h * KH : (h + 1) * KH]
                .T.reshape(KT1, 128, B)
                .transpose(1, 0, 2)
                .astype(np.float32)
            )
            zw_shard = np.ascontiguousarray(
                np.concatenate([z1_blk, w1_halves[h]], axis=2)
            )
            in_maps.append(
                {
                    "stream": streams[(b, h)],
                    "bp": bp_halves[h],
                    "zw": zw_shard,
                }
            )
    return in_maps


def kernel(z1, z2, W_proj, b_proj, W_fc, b_fc):
    global _NC_CACHE, LAST_RESULTS

    z1 = np.asarray(z1, dtype=np.float32)
    z2 = np.asarray(z2, dtype=np.float32)
    W_proj = np.asarray(W_proj, dtype=np.float32)
    b_proj = np.asarray(b_proj, dtype=np.float32)
    W_fc = np.asarray(W_fc, dtype=np.float32)
    b_fc = np.asarray(b_fc, dtype=np.float32)

    if _NC_CACHE is None:
        _NC_CACHE = _build_nc()
    nc = _NC_CACHE

    in_maps = _prep_inputs(z1, z2, W_proj, b_proj, W_fc)
    res = bass_utils.run_bass_kernel_spmd(nc, in_maps, core_ids=list(range(NCORES)))
    LAST_RESULTS = res

    # gather: sum the two K-half s1 partials per batch group, concat groups;
    # colsum chunks concat over h after summing over b.
    A = np.concatenate(
        [
            (
                np.asarray(res.results[2 * b]["s1t"]).astype(np.float64)
                + np.asarray(res.results[2 * b + 1]["s1t"]).astype(np.float64)
            ).T
            for b in range(NB)
        ],
        axis=0,
    )  # [2048, 65], already scaled by BS
    colsum = np.zeros(POUT, dtype=np.float64)
    for h in range(NH):
        acc = np.zeros((128, MT), dtype=np.float64)
        for b in range(NB):
            acc += np.asarray(res.results[2 * b + h]["colsum"]).astype(np.float64)
        colsum[h * MT * 128 : (h + 1) * MT * 128] = acc.T.reshape(-1)
    vec = W_fc[:, HID:].astype(np.float64) @ colsum + np.float64(BS) * b_fc.astype(
        np.float64
    )
    out = A + vec[None, :]
    return out.astype(np.float32)
